# revision 16
# baseline (speedup 1.0000x reference)
"""Multi-head attention (B=384, S=128, E=512, H=4, D=128) on 8 TRN2 NeuronCores.

Data-parallel: batch 384 -> 48 per core, projection weights replicated.

All-16-bit PE pipeline (PSUM accumulation stays fp32): weights are
pre-converted to fp16 on the host (halves the startup weight DMA and kills
all on-chip conversion work), x is converted to fp16 on ACT/DVE before its
PE transpose, and the PSUM evictions cast everything to fp16/bf16 so every
matmul and transpose runs 16-bit at 1 cycle/row (fp32r matmuls with a 128
moving dim run at 2 cycles/row on HW -- ~107ns vs ~53ns measured for the
score matmuls -- and fp32r transposes at 1.5).

Per-core dataflow:

  x16  = fp16(x)                 ACT/DVE halves (GpSimd is useless: ~2.4us
                                 fixed per-op software cost measured)
  xT   = transpose(x16)          PE transpose (fp16), ACT eviction
  QT   = Wq16^T @ xT + bq        [E_out, rows], ACT eviction -> fp16
  KT   = Wk16^T @ xT + bk        [E_out, rows], ACT eviction -> fp16
  V    = xT-chunks @ Wv16 + bv   [rows, E_out], DVE eviction -> bf16
  per batch (4 heads packed along the PSUM free dim):
    S    = qT.T @ kT             [S, H, T] fp16 matmul -> fp32 PSUM
    w    = exp(S) bf16           ACT (no max-sub: |S| < 88 so bf16 exp cannot
                                 overflow; bf16 keeps the fp32 exponent range)
    sum  = reduce_sum(w)         DVE, reciprocal DVE
    wn   = w * (1/rowsum)        DVE (2-byte 2x mode), bf16
    wT   = transpose(wn)         PE transpose (bf16), DVE eviction
    attT = lhsT(v) @ wT          [D, H, S] bf16 matmul, ACT eviction -> fp16
  O    = att @ Wo16 + bo         (lhsT = attT chunk fp16, rhs = Wo16) -> f32

Engine budget (measured warm): PE ~222us busy (projections ~164us = the
1-cyc/row floor for this op mix at >=fp16 precision; 768 small matmuls
~50us at their ~53ns floor), ACT ~154us, DVE ~160us, GpSimd idle.
Rel err 2.8e-3 (gate 2e-2): fp16 x/W quantization in the q/k path gives
~5e-3 worst case, bf16 softmax weights ~3e-3 (validated in numpy).
Run-to-run HW variance is +/-10-20% (chip-wide clock throttle state).

Startup: every DMA dispatch costs ~650ns-2us serially on its issuing
sequencer (descriptor generation), so x loads go on the SP queue while the
(single-instruction-per-matrix, host-packed) weight + bias loads go on the
ACT queue in parallel, and chunk 0/1 conversions+evictions run on the DVE
(the ACT queue is still generating weight descriptors then). ~6 dummy bf16
matmuls ramp the PE HAM clock-gate while the first x tiles land. Steady
state emits scores(k) | projections(k+1) | x-transposes(k+2) | tail(k) so
chunk k's softmax chain drains on ACT/DVE under chunk k+1's projections;
the last two chunks interleave their tails to keep the PE fed through the
drain (tail-O of chunk 10 fills the softmax latency of chunk 11), and the
final stores split across both hwdge queues per batch.
"""

import numpy as np

import concourse.bass as bass
import concourse.tile as tile
import concourse.mybir as mybir
from concourse import bacc
from concourse.bass_utils import run_bass_kernel_spmd
from concourse.masks import make_identity

B, S, E, H, D = 384, 128, 512, 4, 128
NCORES = 8
BLOC = B // NCORES  # 48 batches per core
NB = 4  # batches per chunk
NCHUNK = BLOC // NB
NBS = NB * S  # 512 rows of x per chunk
EC = E // 128  # 4 chunks of the embed dim

F32 = mybir.dt.float32
F32R = mybir.dt.float32r
BF16 = mybir.dt.bfloat16
F16 = mybir.dt.float16

_CACHE = {}


def build():
    nc = bacc.Bacc("TRN2", target_bir_lowering=False, debug=False, num_devices=NCORES)

    x = nc.dram_tensor("x", [BLOC, S, E], F32R, kind="ExternalInput").ap()
    wq16 = nc.dram_tensor("Wq16", [E, E], F16, kind="ExternalInput").ap()
    wk16 = nc.dram_tensor("Wk16", [E, E], F16, kind="ExternalInput").ap()
    wv16 = nc.dram_tensor("Wv16", [E, E], F16, kind="ExternalInput").ap()
    wo16 = nc.dram_tensor("Wo16", [E, E], F16, kind="ExternalInput").ap()
    bqp = nc.dram_tensor("bqp", [128, EC], F32, kind="ExternalInput").ap()
    bkp = nc.dram_tensor("bkp", [128, EC], F32, kind="ExternalInput").ap()
    bv = nc.dram_tensor("bv", [E], F32, kind="ExternalInput").ap()
    bo = nc.dram_tensor("bo", [E], F32, kind="ExternalInput").ap()
    out = nc.dram_tensor("out", [BLOC, S, E], F32, kind="ExternalOutput").ap()

    with tile.TileContext(nc) as tc:
        with (
            tc.tile_pool(name="singles", bufs=1) as singles,
            tc.tile_pool(name="xp", bufs=2) as xp,
            tc.tile_pool(name="qkv", bufs=2) as qkv,
            tc.tile_pool(name="attn", bufs=2) as attn,
            tc.tile_pool(name="wsm", bufs=4) as wsm,
            tc.tile_pool(name="stats", bufs=8) as stats,
            tc.tile_pool(name="ps", bufs=8, space="PSUM") as ps,
        ):
            w16 = {}
            w_dram = {"q": wq16, "k": wk16, "v": wv16, "o": wo16}
            for name in ("q", "k", "v", "o"):
                w16[name] = singles.tile(
                    [128, EC, E], F16, tag=f"w{name}", name=f"w{name}"
                )

            def load_weights():
                # One dispatch per matrix, on the ACT hwdge queue so the SP
                # queue is free for the x loads.
                for name in ("q", "k", "v", "o"):
                    nc.scalar.dma_start(
                        out=w16[name][:, :, :],
                        in_=bass.AP(
                            tensor=w_dram[name].tensor,
                            offset=0,
                            ap=[[E, 128], [128 * E, EC], [1, E]],
                        ),
                    )

            bq_sb = singles.tile([128, EC], F32, tag="bq")
            bk_sb = singles.tile([128, EC], F32, tag="bk")
            bv_sb = singles.tile([128, E], F32, tag="bv")
            bo_sb = singles.tile([128, E], F32, tag="bo")

            def load_biases():
                nc.scalar.dma_start(out=bq_sb, in_=bqp[:, :])
                nc.scalar.dma_start(out=bk_sb, in_=bkp[:, :])
                for t, b in ((bv_sb, bv), (bo_sb, bo)):
                    nc.scalar.dma_start(
                        out=t,
                        in_=bass.AP(tensor=b.tensor, offset=0, ap=[[0, 128], [1, E]]),
                    )

            ident_f16 = singles.tile([128, 128], F16, tag="idf16")
            make_identity(nc, ident_f16[:])
            ident_f32 = singles.tile([128, 128], F32, tag="idf32")
            make_identity(nc, ident_f32[:])
            ident = singles.tile([128, 128], F32R, tag="idf")
            nc.vector.tensor_copy(out=ident, in_=ident_f32[:].bitcast(F32R))
            ident_bf = singles.tile([128, 128], BF16, tag="idb")
            make_identity(nc, ident_bf[:])

            # Warm the PE HAM clock-gate while the first x tiles land (PE
            # would otherwise start cold at half clock).
            dummy_bf = singles.tile([128, E], BF16, tag="dummy")
            nc.vector.memset(dummy_bf, 0.0)
            warm_ps = ps.tile([128, E], F32, tag="ps", name="warm")
            for _ in range(6):
                nc.tensor.matmul(warm_ps, ident_bf[:], dummy_bf, start=True, stop=True)

            def emit_x_dma(chunk, fine=False):
                b0 = chunk * NB
                t = xp.tile([128, NB, E], F32R, tag="xnat")
                if fine:
                    # separate dispatches overlap descriptor generation with
                    # the transfer, getting the first bytes moving sooner
                    for j in range(NB):
                        nc.sync.dma_start(out=t[:, j, :], in_=x[b0 + j])
                else:
                    nc.sync.dma_start(
                        out=t,
                        in_=bass.AP(
                            tensor=x.tensor,
                            offset=b0 * S * E,
                            ap=[[E, 128], [S * E, NB], [1, E]],
                        ),
                    )
                return t

            def conv_trans0(x_nat, by_batch=True):
                """Startup path for chunks 0/1: transpose f32r straight from
                the DMA'd x (no conversion latency -- the PE is DMA-gated
                here anyway), evict as fp16 on the DVE. by_batch orders the
                transposes j-major so PE work starts as soon as batch 0
                lands."""
                psts = [
                    ps.tile([128, NBS], F32R, tag="ps", name=f"pst{c}")
                    for c in range(EC)
                ]
                order = (
                    [(j, c) for j in range(NB) for c in range(EC)]
                    if by_batch
                    else [(j, c) for c in range(EC) for j in range(NB)]
                )
                for j, c in order:
                    nc.tensor.transpose(
                        psts[c][:, j * 128 : (j + 1) * 128],
                        x_nat[:, j, c * 128 : (c + 1) * 128],
                        ident[:],
                    )
                xt = []
                for c in range(EC):
                    t = xp.tile([128, NBS], F16, tag=f"xt{c}")
                    nc.vector.tensor_copy(out=t, in_=psts[c].bitcast(F32))
                    xt.append(t)
                return xt

            def conv16(x_nat):
                """fp16 convert of an x chunk (ACT+DVE halves). Emitted right
                after scores(k) so the DVE half clears its queue before the
                wT evictions that chunk k's att matmuls wait on."""
                x16 = xp.tile([128, NB, E], F16, tag="x16")
                nc.scalar.copy(
                    out=x16[:, 0:2, :], in_=x_nat[:, 0:2, :].bitcast(F32)
                )
                nc.vector.tensor_copy(
                    out=x16[:, 2:4, :], in_=x_nat[:, 2:4, :].bitcast(F32)
                )
                return x16

            def trans16(x16):
                """PE transpose of a pre-converted fp16 x chunk -> xT tiles."""
                xt = []
                for c in range(EC):
                    pst = ps.tile([128, NBS], F16, tag="ps")
                    for j in range(NB):
                        nc.tensor.transpose(
                            pst[:, j * 128 : (j + 1) * 128],
                            x16[:, j, c * 128 : (c + 1) * 128],
                            ident_f16[:],
                        )
                    t = xp.tile([128, NBS], F16, tag=f"xt{c}")
                    nc.scalar.copy(out=t, in_=pst)
                    xt.append(t)
                return xt

            def proj(xt):
                """QT/KT/V projections from xT (fp16 operands, fp32 PSUM)."""
                qt, kt = [], []
                for h in range(H):
                    p = ps.tile([128, NBS], F32, tag="ps")
                    for c in range(EC):
                        nc.tensor.matmul(
                            p,
                            w16["q"][:, c, h * 128 : (h + 1) * 128],
                            xt[c],
                            start=(c == 0),
                            stop=(c == EC - 1),
                        )
                    t = qkv.tile([128, NBS], F16, tag=f"qt{h}")
                    nc.scalar.add(out=t, in_=p, add=bq_sb[:, h : h + 1])
                    qt.append(t)
                    p = ps.tile([128, NBS], F32, tag="ps")
                    for c in range(EC):
                        nc.tensor.matmul(
                            p,
                            w16["k"][:, c, h * 128 : (h + 1) * 128],
                            xt[c],
                            start=(c == 0),
                            stop=(c == EC - 1),
                        )
                    t = qkv.tile([128, NBS], F16, tag=f"kt{h}")
                    nc.scalar.add(out=t, in_=p, add=bk_sb[:, h : h + 1])
                    kt.append(t)
                v_sb = []
                for j in range(NB):
                    p = ps.tile([128, E], F32, tag="ps")
                    for c in range(EC):
                        nc.tensor.matmul(
                            p,
                            xt[c][:, j * 128 : (j + 1) * 128],
                            w16["v"][:, c, :],
                            start=(c == 0),
                            stop=(c == EC - 1),
                        )
                    t = qkv.tile([128, E], BF16, tag=f"v{j}")
                    nc.vector.tensor_add(out=t, in0=p, in1=bv_sb)
                    v_sb.append(t)
                return qt, kt, v_sb

            def attn_scores(qt, kt, js=None, act_norms=False):
                """scores + softmax (no max-subtraction) -> normalized bf16 w."""
                wns = []
                for j in (range(NB) if js is None else js):
                    ps_s = ps.tile([128, H, 128], F32, tag="ps")
                    for h in range(H):
                        nc.tensor.matmul(
                            ps_s[:, h, :],
                            qt[h][:, j * 128 : (j + 1) * 128],
                            kt[h][:, j * 128 : (j + 1) * 128],
                            start=True,
                            stop=True,
                        )
                    w_exp = wsm.tile([128, H, 128], BF16, tag="wexp")
                    nc.scalar.activation(
                        out=w_exp,
                        in_=ps_s,
                        func=mybir.ActivationFunctionType.Exp,
                        bias=0.0,
                        scale=1.0,
                    )
                    sumexp = stats.tile([128, H], F32, tag="sumexp")
                    nc.vector.reduce_sum(
                        out=sumexp, in_=w_exp, axis=mybir.AxisListType.X
                    )
                    recip = stats.tile([128, H], F32, tag="recip")
                    nc.vector.reciprocal(out=recip, in_=sumexp)
                    wn = wsm.tile([128, H, 128], BF16, tag="wn")
                    for h in range(H):
                        if act_norms and h < 2:
                            nc.scalar.activation(
                                out=wn[:, h, :],
                                in_=w_exp[:, h, :],
                                func=mybir.ActivationFunctionType.Identity,
                                bias=0.0,
                                scale=recip[:, h : h + 1],
                            )
                        else:
                            nc.vector.tensor_scalar_mul(
                                out=wn[:, h, :],
                                in0=w_exp[:, h, :],
                                scalar1=recip[:, h : h + 1],
                            )
                    wns.append(wn)
                return wns

            def tail_watt(chunk, wns, v_sb, js=None):
                """wT transposes + att = v.T-form matmuls -> at fp16 tiles."""
                js = range(NB) if js is None else js
                wn_of = wns if callable(wns) else lambda j: wns[j]
                wt_sbs = {}
                for j in js:
                    ps_wt = ps.tile([128, H, 128], BF16, tag="ps")
                    for h in range(H):
                        nc.tensor.transpose(
                            ps_wt[:, h, :], wn_of(j)[:, h, :], ident_bf[:]
                        )
                    wt_sb = wsm.tile([128, H, 128], BF16, tag=f"wt{j}")
                    nc.vector.tensor_copy(out=wt_sb, in_=ps_wt)
                    wt_sbs[j] = wt_sb
                ats = {}
                for j in js:
                    ps_at = ps.tile([128, H, 128], F32, tag="ps")
                    for h in range(H):
                        nc.tensor.matmul(
                            ps_at[:, h, :],
                            v_sb[j][:, h * 128 : (h + 1) * 128],
                            wt_sbs[j][:, h, :],
                            start=True,
                            stop=True,
                        )
                    at = attn.tile([128, H, 128], F16, tag=f"at{j}")
                    nc.scalar.copy(out=at, in_=ps_at)
                    ats[j] = at
                return ats

            def tail_o(chunk, ats, js=None, o_sb=None):
                """O projection + bias + store."""
                b0 = chunk * NB
                js = range(NB) if js is None else js
                split_store = chunk == NCHUNK - 1
                if o_sb is None:
                    o_sb = attn.tile([128, NB, E], F32, tag="o")
                for j in js:
                    p = ps.tile([128, E], F32, tag="ps")
                    for h in range(H):
                        nc.tensor.matmul(
                            p,
                            ats[j][:, h, :],
                            w16["o"][:, h, :],
                            start=(h == 0),
                            stop=(h == H - 1),
                        )
                    nc.vector.tensor_add(out=o_sb[:, j, :], in0=p, in1=bo_sb)
                    if split_store:
                        eng = nc.sync if j % 2 == 0 else nc.scalar
                        eng.dma_start(out=out[b0 + j], in_=o_sb[:, j, :])
                if not split_store and (NB - 1) in js:
                    nc.sync.dma_start(
                        out=bass.AP(
                            tensor=out.tensor,
                            offset=b0 * S * E,
                            ap=[[E, 128], [S * E, NB], [1, E]],
                        ),
                        in_=o_sb,
                    )
                return o_sb

            # Startup: x0/x1 dispatch on the SP queue, weights+biases on the
            # ACT queue (emitted first so their dispatch overlaps the x DMA),
            # then the software pipeline:
            #   scores(k) | projections(k+1) | transposes(k+2) | tail(k)
            x0 = emit_x_dma(0, fine=True)
            x1 = emit_x_dma(1)
            load_weights()
            load_biases()
            xts = {0: conv_trans0(x0)}
            states = {0: proj(xts[0])}
            xts[1] = conv_trans0(x1, by_batch=False)
            wns = {}
            for k in range(NCHUNK - 1):
                wns[k] = attn_scores(states[k][0], states[k][1])
                if k + 2 < NCHUNK:
                    x16_next = conv16(emit_x_dma(k + 2))
                states[k + 1] = proj(xts[k + 1])
                if k + 2 < NCHUNK:
                    xts[k + 2] = trans16(x16_next)
                ats = tail_watt(k, wns[k], states[k][2])
                if k < NCHUNK - 2:
                    tail_o(k, ats)
                else:
                    o_prev = tail_o(k, ats, js=[0, 1])
                    ats_prev = ats
            kl = NCHUNK - 1
            wns[kl] = attn_scores(states[kl][0], states[kl][1], act_norms=True)
            tail_o(kl - 1, ats_prev, js=[2, 3], o_sb=o_prev)
            ats = tail_watt(kl, wns[kl], states[kl][2])
            tail_o(kl, ats)

    nc.compile()
    return nc


def kernel(**inputs):
    if "nc" not in _CACHE:
        _CACHE["nc"] = build()
    nc = _CACHE["nc"]

    x = np.ascontiguousarray(np.asarray(inputs["x"], dtype=np.float32))
    shared = {
        "Wq16": np.ascontiguousarray(np.asarray(inputs["Wq"], dtype=np.float16)),
        "Wk16": np.ascontiguousarray(np.asarray(inputs["Wk"], dtype=np.float16)),
        "Wv16": np.ascontiguousarray(np.asarray(inputs["Wv"], dtype=np.float16)),
        "Wo16": np.ascontiguousarray(np.asarray(inputs["Wo"], dtype=np.float16)),
        "bqp": np.ascontiguousarray(
            np.asarray(inputs["bq"], dtype=np.float32).reshape(EC, 128).T
        ),
        "bkp": np.ascontiguousarray(
            np.asarray(inputs["bk"], dtype=np.float32).reshape(EC, 128).T
        ),
        "bv": np.ascontiguousarray(np.asarray(inputs["bv"], dtype=np.float32)),
        "bo": np.ascontiguousarray(np.asarray(inputs["bo"], dtype=np.float32)),
    }
    in_maps = [
        {"x": x[i * BLOC : (i + 1) * BLOC], **shared} for i in range(NCORES)
    ]
    res = run_bass_kernel_spmd(nc, in_maps, core_ids=list(range(NCORES)))
    return np.concatenate([res.results[i]["out"] for i in range(NCORES)], axis=0)


# revision 17
# speedup vs baseline: 1.0014x; 1.0014x over previous
"""Multi-head attention (B=384, S=128, E=512, H=4, D=128) on 8 TRN2 NeuronCores.

Data-parallel: batch 384 -> 48 per core, projection weights replicated.

All-16-bit PE pipeline (PSUM accumulation stays fp32): weights are
pre-converted to fp16 on the host (halves the startup weight DMA and kills
all on-chip conversion work), x is converted to fp16 on ACT/DVE before its
PE transpose, and the PSUM evictions cast everything to fp16/bf16 so every
matmul and transpose runs 16-bit at 1 cycle/row (fp32r matmuls with a 128
moving dim run at 2 cycles/row on HW -- ~107ns vs ~53ns measured for the
score matmuls -- and fp32r transposes at 1.5).

Per-core dataflow:

  x16  = fp16(x)                 ACT/DVE halves (GpSimd is useless: ~2.4us
                                 fixed per-op software cost measured)
  xT   = transpose(x16)          PE transpose (fp16), ACT eviction
  QT   = Wq16^T @ xT + bq        [E_out, rows], ACT eviction -> fp16
  KT   = Wk16^T @ xT + bk        [E_out, rows], ACT eviction -> fp16
  V    = xT-chunks @ Wv16 + bv   [rows, E_out], DVE eviction -> bf16
  per batch (4 heads packed along the PSUM free dim):
    S    = qT.T @ kT             [S, H, T] fp16 matmul -> fp32 PSUM
    w    = exp(S) bf16           ACT (no max-sub: |S| < 88 so bf16 exp cannot
                                 overflow; bf16 keeps the fp32 exponent range)
    sum  = reduce_sum(w)         DVE, reciprocal DVE
    wn   = w * (1/rowsum)        DVE (2-byte 2x mode), bf16
    wT   = transpose(wn)         PE transpose (bf16), DVE eviction
    attT = lhsT(v) @ wT          [D, H, S] bf16 matmul, ACT eviction -> fp16
  O    = att @ Wo16 + bo         (lhsT = attT chunk fp16, rhs = Wo16) -> f32

Engine budget (measured warm): PE ~222us busy (projections ~164us = the
1-cyc/row floor for this op mix at >=fp16 precision; 768 small matmuls
~50us at their ~53ns floor), ACT ~154us, DVE ~160us, GpSimd idle.
Rel err 2.8e-3 (gate 2e-2): fp16 x/W quantization in the q/k path gives
~5e-3 worst case, bf16 softmax weights ~3e-3 (validated in numpy).
Run-to-run HW variance is +/-10-20% (chip-wide clock throttle state).

Startup: every DMA dispatch costs ~650ns-2us serially on its issuing
sequencer (descriptor generation), so x loads go on the SP queue while the
(single-instruction-per-matrix, host-packed) weight + bias loads go on the
ACT queue in parallel, and chunk 0/1 conversions+evictions run on the DVE
(the ACT queue is still generating weight descriptors then). ~6 dummy bf16
matmuls ramp the PE HAM clock-gate while the first x tiles land. Steady
state emits scores(k) | projections(k+1) | x-transposes(k+2) | tail(k) so
chunk k's softmax chain drains on ACT/DVE under chunk k+1's projections;
the last two chunks interleave their tails to keep the PE fed through the
drain (tail-O of chunk 10 fills the softmax latency of chunk 11), and the
final stores split across both hwdge queues per batch.
"""

import numpy as np

import concourse.bass as bass
import concourse.tile as tile
import concourse.mybir as mybir
from concourse import bacc
from concourse.bass_utils import run_bass_kernel_spmd
from concourse.masks import make_identity

B, S, E, H, D = 384, 128, 512, 4, 128
NCORES = 8
BLOC = B // NCORES  # 48 batches per core
NB = 4  # batches per chunk
NCHUNK = BLOC // NB
NBS = NB * S  # 512 rows of x per chunk
EC = E // 128  # 4 chunks of the embed dim

F32 = mybir.dt.float32
F32R = mybir.dt.float32r
BF16 = mybir.dt.bfloat16
F16 = mybir.dt.float16

_CACHE = {}


def build():
    nc = bacc.Bacc("TRN2", target_bir_lowering=False, debug=False, num_devices=NCORES)

    x = nc.dram_tensor("x", [BLOC, S, E], F32R, kind="ExternalInput").ap()
    wq16 = nc.dram_tensor("Wq16", [E, E], F16, kind="ExternalInput").ap()
    wk16 = nc.dram_tensor("Wk16", [E, E], F16, kind="ExternalInput").ap()
    wv16 = nc.dram_tensor("Wv16", [E, E], F16, kind="ExternalInput").ap()
    wo16 = nc.dram_tensor("Wo16", [E, E], F16, kind="ExternalInput").ap()
    bqp = nc.dram_tensor("bqp", [128, EC], F32, kind="ExternalInput").ap()
    bkp = nc.dram_tensor("bkp", [128, EC], F32, kind="ExternalInput").ap()
    bv = nc.dram_tensor("bv", [E], F32, kind="ExternalInput").ap()
    bo = nc.dram_tensor("bo", [E], F32, kind="ExternalInput").ap()
    out = nc.dram_tensor("out", [BLOC, S, E], F32, kind="ExternalOutput").ap()

    with tile.TileContext(nc) as tc:
        with (
            tc.tile_pool(name="singles", bufs=1) as singles,
            tc.tile_pool(name="xp", bufs=2) as xp,
            tc.tile_pool(name="qkv", bufs=2) as qkv,
            tc.tile_pool(name="attn", bufs=2) as attn,
            tc.tile_pool(name="wsm", bufs=4) as wsm,
            tc.tile_pool(name="stats", bufs=8) as stats,
            tc.tile_pool(name="ps", bufs=8, space="PSUM") as ps,
        ):
            w16 = {}
            w_dram = {"q": wq16, "k": wk16, "v": wv16, "o": wo16}
            for name in ("q", "k", "v", "o"):
                w16[name] = singles.tile(
                    [128, EC, E], F16, tag=f"w{name}", name=f"w{name}"
                )

            def load_weights():
                # One dispatch per matrix, on the ACT hwdge queue so the SP
                # queue is free for the x loads.
                for name in ("q", "k", "v", "o"):
                    nc.scalar.dma_start(
                        out=w16[name][:, :, :],
                        in_=bass.AP(
                            tensor=w_dram[name].tensor,
                            offset=0,
                            ap=[[E, 128], [128 * E, EC], [1, E]],
                        ),
                    )

            bq_sb = singles.tile([128, EC], F32, tag="bq")
            bk_sb = singles.tile([128, EC], F32, tag="bk")
            bv_sb = singles.tile([128, E], F32, tag="bv")
            bo_sb = singles.tile([128, E], F32, tag="bo")

            def load_biases():
                nc.scalar.dma_start(out=bq_sb, in_=bqp[:, :])
                nc.scalar.dma_start(out=bk_sb, in_=bkp[:, :])
                for t, b in ((bv_sb, bv), (bo_sb, bo)):
                    nc.scalar.dma_start(
                        out=t,
                        in_=bass.AP(tensor=b.tensor, offset=0, ap=[[0, 128], [1, E]]),
                    )

            ident_f16 = singles.tile([128, 128], F16, tag="idf16")
            make_identity(nc, ident_f16[:])
            ident_f32 = singles.tile([128, 128], F32, tag="idf32")
            make_identity(nc, ident_f32[:])
            ident = singles.tile([128, 128], F32R, tag="idf")
            nc.vector.tensor_copy(out=ident, in_=ident_f32[:].bitcast(F32R))
            ident_bf = singles.tile([128, 128], BF16, tag="idb")
            make_identity(nc, ident_bf[:])

            # Warm the PE HAM clock-gate while the first x tiles land (PE
            # would otherwise start cold at half clock).
            dummy_bf = singles.tile([128, E], BF16, tag="dummy")
            nc.vector.memset(dummy_bf, 0.0)
            warm_ps = ps.tile([128, E], F32, tag="ps", name="warm")
            for _ in range(6):
                nc.tensor.matmul(warm_ps, ident_bf[:], dummy_bf, start=True, stop=True)

            def emit_x_dma(chunk, fine=False):
                b0 = chunk * NB
                t = xp.tile([128, NB, E], F32R, tag="xnat")
                if fine:
                    # separate dispatches overlap descriptor generation with
                    # the transfer, getting the first bytes moving sooner
                    for j in range(NB):
                        nc.sync.dma_start(out=t[:, j, :], in_=x[b0 + j])
                else:
                    nc.sync.dma_start(
                        out=t,
                        in_=bass.AP(
                            tensor=x.tensor,
                            offset=b0 * S * E,
                            ap=[[E, 128], [S * E, NB], [1, E]],
                        ),
                    )
                return t

            def conv_trans0(x_nat, by_batch=True):
                """Startup path for chunks 0/1: transpose f32r straight from
                the DMA'd x (no conversion latency -- the PE is DMA-gated
                here anyway), evict as fp16 on the DVE. by_batch orders the
                transposes j-major so PE work starts as soon as batch 0
                lands."""
                psts = [
                    ps.tile([128, NBS], F32R, tag="ps", name=f"pst{c}")
                    for c in range(EC)
                ]
                order = (
                    [(j, c) for j in range(NB) for c in range(EC)]
                    if by_batch
                    else [(j, c) for c in range(EC) for j in range(NB)]
                )
                for j, c in order:
                    nc.tensor.transpose(
                        psts[c][:, j * 128 : (j + 1) * 128],
                        x_nat[:, j, c * 128 : (c + 1) * 128],
                        ident[:],
                    )
                xt = []
                for c in range(EC):
                    t = xp.tile([128, NBS], F16, tag=f"xt{c}")
                    nc.vector.tensor_copy(out=t, in_=psts[c].bitcast(F32))
                    xt.append(t)
                return xt

            def conv16(x_nat):
                """fp16 convert of an x chunk (ACT+DVE halves). Emitted right
                after scores(k) so the DVE half clears its queue before the
                wT evictions that chunk k's att matmuls wait on."""
                x16 = xp.tile([128, NB, E], F16, tag="x16")
                nc.scalar.copy(
                    out=x16[:, 0:2, :], in_=x_nat[:, 0:2, :].bitcast(F32)
                )
                nc.vector.tensor_copy(
                    out=x16[:, 2:4, :], in_=x_nat[:, 2:4, :].bitcast(F32)
                )
                return x16

            def trans16(x16):
                """PE transpose of a pre-converted fp16 x chunk -> xT tiles."""
                xt = []
                for c in range(EC):
                    pst = ps.tile([128, NBS], F16, tag="ps")
                    for j in range(NB):
                        nc.tensor.transpose(
                            pst[:, j * 128 : (j + 1) * 128],
                            x16[:, j, c * 128 : (c + 1) * 128],
                            ident_f16[:],
                        )
                    t = xp.tile([128, NBS], F16, tag=f"xt{c}")
                    nc.scalar.copy(out=t, in_=pst)
                    xt.append(t)
                return xt

            def proj(xt):
                """QT/KT/V projections from xT (fp16 operands, fp32 PSUM)."""
                qt, kt = [], []
                for h in range(H):
                    p = ps.tile([128, NBS], F32, tag="ps")
                    for c in range(EC):
                        nc.tensor.matmul(
                            p,
                            w16["q"][:, c, h * 128 : (h + 1) * 128],
                            xt[c],
                            start=(c == 0),
                            stop=(c == EC - 1),
                        )
                    t = qkv.tile([128, NBS], F16, tag=f"qt{h}")
                    nc.scalar.add(out=t, in_=p, add=bq_sb[:, h : h + 1])
                    qt.append(t)
                    p = ps.tile([128, NBS], F32, tag="ps")
                    for c in range(EC):
                        nc.tensor.matmul(
                            p,
                            w16["k"][:, c, h * 128 : (h + 1) * 128],
                            xt[c],
                            start=(c == 0),
                            stop=(c == EC - 1),
                        )
                    t = qkv.tile([128, NBS], F16, tag=f"kt{h}")
                    nc.scalar.add(out=t, in_=p, add=bk_sb[:, h : h + 1])
                    kt.append(t)
                v_sb = []
                for j in range(NB):
                    p = ps.tile([128, E], F32, tag="ps")
                    for c in range(EC):
                        nc.tensor.matmul(
                            p,
                            xt[c][:, j * 128 : (j + 1) * 128],
                            w16["v"][:, c, :],
                            start=(c == 0),
                            stop=(c == EC - 1),
                        )
                    t = qkv.tile([128, E], BF16, tag=f"v{j}")
                    nc.vector.tensor_add(out=t, in0=p, in1=bv_sb)
                    v_sb.append(t)
                return qt, kt, v_sb

            def attn_scores(qt, kt, js=None, act_norms=False):
                """scores + softmax (no max-subtraction) -> normalized bf16 w."""
                wns = []
                for j in (range(NB) if js is None else js):
                    ps_s = ps.tile([128, H, 128], F32, tag="ps")
                    for h in range(H):
                        nc.tensor.matmul(
                            ps_s[:, h, :],
                            qt[h][:, j * 128 : (j + 1) * 128],
                            kt[h][:, j * 128 : (j + 1) * 128],
                            start=True,
                            stop=True,
                        )
                    w_exp = wsm.tile([128, H, 128], BF16, tag="wexp")
                    nc.scalar.activation(
                        out=w_exp,
                        in_=ps_s,
                        func=mybir.ActivationFunctionType.Exp,
                        bias=0.0,
                        scale=1.0,
                    )
                    sumexp = stats.tile([128, H], F32, tag="sumexp")
                    nc.vector.reduce_sum(
                        out=sumexp, in_=w_exp, axis=mybir.AxisListType.X
                    )
                    recip = stats.tile([128, H], F32, tag="recip")
                    nc.vector.reciprocal(out=recip, in_=sumexp)
                    wn = wsm.tile([128, H, 128], BF16, tag="wn")
                    for h in range(H):
                        if act_norms and h < 2:
                            nc.scalar.activation(
                                out=wn[:, h, :],
                                in_=w_exp[:, h, :],
                                func=mybir.ActivationFunctionType.Identity,
                                bias=0.0,
                                scale=recip[:, h : h + 1],
                            )
                        else:
                            nc.vector.tensor_scalar_mul(
                                out=wn[:, h, :],
                                in0=w_exp[:, h, :],
                                scalar1=recip[:, h : h + 1],
                            )
                    wns.append(wn)
                return wns

            def tail_watt(chunk, wns, v_sb, js=None):
                """wT transposes + att = v.T-form matmuls -> at fp16 tiles."""
                js = range(NB) if js is None else js
                wn_of = wns if callable(wns) else lambda j: wns[j]
                wt_sbs = {}
                for j in js:
                    ps_wt = ps.tile([128, H, 128], BF16, tag="ps")
                    for h in range(H):
                        nc.tensor.transpose(
                            ps_wt[:, h, :], wn_of(j)[:, h, :], ident_bf[:]
                        )
                    wt_sb = wsm.tile([128, H, 128], BF16, tag=f"wt{j}")
                    nc.vector.tensor_copy(out=wt_sb, in_=ps_wt)
                    wt_sbs[j] = wt_sb
                ats = {}
                for j in js:
                    ps_at = ps.tile([128, H, 128], F32, tag="ps")
                    for h in range(H):
                        nc.tensor.matmul(
                            ps_at[:, h, :],
                            v_sb[j][:, h * 128 : (h + 1) * 128],
                            wt_sbs[j][:, h, :],
                            start=True,
                            stop=True,
                        )
                    at = attn.tile([128, H, 128], F16, tag=f"at{j}")
                    nc.scalar.copy(out=at, in_=ps_at)
                    ats[j] = at
                return ats

            def tail_o(chunk, ats, js=None, o_sb=None):
                """O projection + bias + store."""
                b0 = chunk * NB
                js = range(NB) if js is None else js
                split_store = chunk == NCHUNK - 1
                if o_sb is None:
                    o_sb = attn.tile([128, NB, E], F32, tag="o")
                for j in js:
                    p = ps.tile([128, E], F32, tag="ps")
                    for h in range(H):
                        nc.tensor.matmul(
                            p,
                            ats[j][:, h, :],
                            w16["o"][:, h, :],
                            start=(h == 0),
                            stop=(h == H - 1),
                        )
                    nc.vector.tensor_add(out=o_sb[:, j, :], in0=p, in1=bo_sb)
                    if split_store:
                        eng = nc.sync if j % 2 == 0 else nc.scalar
                        eng.dma_start(out=out[b0 + j], in_=o_sb[:, j, :])
                if not split_store and (NB - 1) in js:
                    nc.sync.dma_start(
                        out=bass.AP(
                            tensor=out.tensor,
                            offset=b0 * S * E,
                            ap=[[E, 128], [S * E, NB], [1, E]],
                        ),
                        in_=o_sb,
                    )
                return o_sb

            # Startup: x0/x1 dispatch on the SP queue, weights+biases on the
            # ACT queue (emitted first so their dispatch overlaps the x DMA),
            # then the software pipeline:
            #   scores(k) | projections(k+1) | transposes(k+2) | tail(k)
            x0 = emit_x_dma(0, fine=True)
            x1 = emit_x_dma(1)
            load_weights()
            load_biases()
            xts = {0: conv_trans0(x0)}
            states = {0: proj(xts[0])}
            xts[1] = conv_trans0(x1, by_batch=False)
            wns = {}
            for k in range(NCHUNK - 1):
                wns[k] = attn_scores(states[k][0], states[k][1])
                if k + 2 < NCHUNK:
                    x16_next = conv16(emit_x_dma(k + 2))
                states[k + 1] = proj(xts[k + 1])
                ats = tail_watt(k, wns[k], states[k][2])
                if k + 2 < NCHUNK:
                    xts[k + 2] = trans16(x16_next)
                if k < NCHUNK - 2:
                    tail_o(k, ats)
                else:
                    o_prev = tail_o(k, ats, js=[0, 1])
                    ats_prev = ats
            kl = NCHUNK - 1
            wns[kl] = attn_scores(states[kl][0], states[kl][1], act_norms=True)
            tail_o(kl - 1, ats_prev, js=[2, 3], o_sb=o_prev)
            ats = tail_watt(kl, wns[kl], states[kl][2])
            tail_o(kl, ats)

    nc.compile()
    return nc


def kernel(**inputs):
    if "nc" not in _CACHE:
        _CACHE["nc"] = build()
    nc = _CACHE["nc"]

    x = np.ascontiguousarray(np.asarray(inputs["x"], dtype=np.float32))
    shared = {
        "Wq16": np.ascontiguousarray(np.asarray(inputs["Wq"], dtype=np.float16)),
        "Wk16": np.ascontiguousarray(np.asarray(inputs["Wk"], dtype=np.float16)),
        "Wv16": np.ascontiguousarray(np.asarray(inputs["Wv"], dtype=np.float16)),
        "Wo16": np.ascontiguousarray(np.asarray(inputs["Wo"], dtype=np.float16)),
        "bqp": np.ascontiguousarray(
            np.asarray(inputs["bq"], dtype=np.float32).reshape(EC, 128).T
        ),
        "bkp": np.ascontiguousarray(
            np.asarray(inputs["bk"], dtype=np.float32).reshape(EC, 128).T
        ),
        "bv": np.ascontiguousarray(np.asarray(inputs["bv"], dtype=np.float32)),
        "bo": np.ascontiguousarray(np.asarray(inputs["bo"], dtype=np.float32)),
    }
    in_maps = [
        {"x": x[i * BLOC : (i + 1) * BLOC], **shared} for i in range(NCORES)
    ]
    res = run_bass_kernel_spmd(nc, in_maps, core_ids=list(range(NCORES)))
    return np.concatenate([res.results[i]["out"] for i in range(NCORES)], axis=0)


# revision 18
# speedup vs baseline: 1.0100x; 1.0086x over previous
"""Multi-head attention (B=384, S=128, E=512, H=4, D=128) on 8 TRN2 NeuronCores.

Data-parallel: batch 384 -> 48 per core, projection weights replicated.

All-16-bit PE pipeline (PSUM accumulation stays fp32): weights are
pre-converted to fp16 on the host (halves the startup weight DMA and kills
all on-chip conversion work), x is converted to fp16 on ACT/DVE before its
PE transpose, and the PSUM evictions cast everything to fp16/bf16 so every
matmul and transpose runs 16-bit at 1 cycle/row (fp32r matmuls with a 128
moving dim run at 2 cycles/row on HW -- ~107ns vs ~53ns measured for the
score matmuls -- and fp32r transposes at 1.5).

Per-core dataflow:

  x16  = fp16(x)                 ACT/DVE halves (GpSimd is useless: ~2.4us
                                 fixed per-op software cost measured)
  xT   = transpose(x16)          PE transpose (fp16), ACT eviction
  QT   = Wq16^T @ xT + bq        [E_out, rows], ACT eviction -> fp16
  KT   = Wk16^T @ xT + bk        [E_out, rows], ACT eviction -> fp16
  V    = xT-chunks @ Wv16 + bv   [rows, E_out], DVE eviction -> bf16
  per batch (4 heads packed along the PSUM free dim):
    S    = qT.T @ kT             [S, H, T] fp16 matmul -> fp32 PSUM
    w    = exp(S) bf16           ACT (no max-sub: |S| < 88 so bf16 exp cannot
                                 overflow; bf16 keeps the fp32 exponent range)
    sum  = reduce_sum(w)         DVE, reciprocal DVE
    wn   = w * (1/rowsum)        DVE (2-byte 2x mode), bf16
    wT   = transpose(wn)         PE transpose (bf16), DVE eviction
    attT = lhsT(v) @ wT          [D, H, S] bf16 matmul, ACT eviction -> fp16
  O    = att @ Wo16 + bo         (lhsT = attT chunk fp16, rhs = Wo16) -> f32

Engine budget (measured warm): PE ~222us busy (projections ~164us = the
1-cyc/row floor for this op mix at >=fp16 precision; 768 small matmuls
~50us at their ~53ns floor), ACT ~154us, DVE ~160us, GpSimd idle.
Rel err 2.8e-3 (gate 2e-2): fp16 x/W quantization in the q/k path gives
~5e-3 worst case, bf16 softmax weights ~3e-3 (validated in numpy).
Run-to-run HW variance is +/-10-20% (chip-wide clock throttle state).

Startup: every DMA dispatch costs ~650ns-2us serially on its issuing
sequencer (descriptor generation), so x loads go on the SP queue while the
(single-instruction-per-matrix, host-packed) weight + bias loads go on the
ACT queue in parallel. Chunks 0/1 transpose f32r straight from the DMA'd x
(no conversion latency; the PE is DMA-gated there anyway) with DVE
evictions, since the ACT queue is still generating weight descriptors
then. ~6 dummy bf16 matmuls ramp the PE HAM clock-gate while the first x
tiles land; PE real work starts ~11us (x0 arrival is the floor). Steady
state emits scores(k) | x-dma+fp16-convert(k+2) | projections(k+1) |
wT/att(k) | x-transposes(k+2) | O(k) so chunk k's softmax chain drains on
ACT/DVE under chunk k+1's projections; the last two chunks interleave
their tails to keep the PE fed through the drain (tail-O of chunk 10
fills the softmax latency of chunk 11), and the final stores split across
both hwdge queues per batch.
"""

import numpy as np

import concourse.bass as bass
import concourse.tile as tile
import concourse.mybir as mybir
from concourse import bacc
from concourse.bass_utils import run_bass_kernel_spmd
from concourse.masks import make_identity

B, S, E, H, D = 384, 128, 512, 4, 128
NCORES = 8
BLOC = B // NCORES  # 48 batches per core
NB = 4  # batches per chunk
NCHUNK = BLOC // NB
NBS = NB * S  # 512 rows of x per chunk
EC = E // 128  # 4 chunks of the embed dim

F32 = mybir.dt.float32
F32R = mybir.dt.float32r
BF16 = mybir.dt.bfloat16
F16 = mybir.dt.float16

_CACHE = {}


def build():
    nc = bacc.Bacc("TRN2", target_bir_lowering=False, debug=False, num_devices=NCORES)

    x = nc.dram_tensor("x", [BLOC, S, E], F32R, kind="ExternalInput").ap()
    wq16 = nc.dram_tensor("Wq16", [E, E], F16, kind="ExternalInput").ap()
    wk16 = nc.dram_tensor("Wk16", [E, E], F16, kind="ExternalInput").ap()
    wv16 = nc.dram_tensor("Wv16", [E, E], F16, kind="ExternalInput").ap()
    wo16 = nc.dram_tensor("Wo16", [E, E], F16, kind="ExternalInput").ap()
    bqp = nc.dram_tensor("bqp", [128, EC], F32, kind="ExternalInput").ap()
    bkp = nc.dram_tensor("bkp", [128, EC], F32, kind="ExternalInput").ap()
    bv = nc.dram_tensor("bv", [E], F32, kind="ExternalInput").ap()
    bo = nc.dram_tensor("bo", [E], F32, kind="ExternalInput").ap()
    out = nc.dram_tensor("out", [BLOC, S, E], F32, kind="ExternalOutput").ap()

    with tile.TileContext(nc) as tc:
        with (
            tc.tile_pool(name="singles", bufs=1) as singles,
            tc.tile_pool(name="xp", bufs=2) as xp,
            tc.tile_pool(name="qkv", bufs=2) as qkv,
            tc.tile_pool(name="attn", bufs=2) as attn,
            tc.tile_pool(name="wsm", bufs=4) as wsm,
            tc.tile_pool(name="stats", bufs=8) as stats,
            tc.tile_pool(name="ps", bufs=8, space="PSUM") as ps,
        ):
            w16 = {}
            w_dram = {"q": wq16, "k": wk16, "v": wv16, "o": wo16}
            for name in ("q", "k", "v", "o"):
                w16[name] = singles.tile(
                    [128, EC, E], F16, tag=f"w{name}", name=f"w{name}"
                )

            def load_weights():
                # One dispatch per matrix, on the ACT hwdge queue so the SP
                # queue is free for the x loads.
                for name in ("q", "k", "v", "o"):
                    nc.scalar.dma_start(
                        out=w16[name][:, :, :],
                        in_=bass.AP(
                            tensor=w_dram[name].tensor,
                            offset=0,
                            ap=[[E, 128], [128 * E, EC], [1, E]],
                        ),
                    )

            bq_sb = singles.tile([128, EC], F32, tag="bq")
            bk_sb = singles.tile([128, EC], F32, tag="bk")
            bv_sb = singles.tile([128, E], F32, tag="bv")
            bo_sb = singles.tile([128, E], F32, tag="bo")

            def load_biases():
                nc.scalar.dma_start(out=bq_sb, in_=bqp[:, :])
                nc.scalar.dma_start(out=bk_sb, in_=bkp[:, :])
                for t, b in ((bv_sb, bv), (bo_sb, bo)):
                    nc.scalar.dma_start(
                        out=t,
                        in_=bass.AP(tensor=b.tensor, offset=0, ap=[[0, 128], [1, E]]),
                    )

            ident_f16 = singles.tile([128, 128], F16, tag="idf16")
            make_identity(nc, ident_f16[:])
            ident_f32 = singles.tile([128, 128], F32, tag="idf32")
            make_identity(nc, ident_f32[:])
            ident = singles.tile([128, 128], F32R, tag="idf")
            nc.vector.tensor_copy(out=ident, in_=ident_f32[:].bitcast(F32R))
            ident_bf = singles.tile([128, 128], BF16, tag="idb")
            make_identity(nc, ident_bf[:])

            # Warm the PE HAM clock-gate while the first x tiles land (PE
            # would otherwise start cold at half clock).
            dummy_bf = singles.tile([128, E], BF16, tag="dummy")
            nc.vector.memset(dummy_bf, 0.0)
            warm_ps = ps.tile([128, E], F32, tag="ps", name="warm")
            for _ in range(6):
                nc.tensor.matmul(warm_ps, ident_bf[:], dummy_bf, start=True, stop=True)

            def emit_x_dma(chunk, fine=False):
                b0 = chunk * NB
                t = xp.tile([128, NB, E], F32R, tag="xnat")
                if fine:
                    # separate dispatches overlap descriptor generation with
                    # the transfer, getting the first bytes moving sooner
                    for j in range(NB):
                        nc.sync.dma_start(out=t[:, j, :], in_=x[b0 + j])
                else:
                    nc.sync.dma_start(
                        out=t,
                        in_=bass.AP(
                            tensor=x.tensor,
                            offset=b0 * S * E,
                            ap=[[E, 128], [S * E, NB], [1, E]],
                        ),
                    )
                return t

            def conv_trans0(x_nat, by_batch=True):
                """Startup path for chunks 0/1: transpose f32r straight from
                the DMA'd x (no conversion latency -- the PE is DMA-gated
                here anyway), evict as fp16 on the DVE. by_batch orders the
                transposes j-major so PE work starts as soon as batch 0
                lands."""
                psts = [
                    ps.tile([128, NBS], F32R, tag="ps", name=f"pst{c}")
                    for c in range(EC)
                ]
                order = (
                    [(j, c) for j in range(NB) for c in range(EC)]
                    if by_batch
                    else [(j, c) for c in range(EC) for j in range(NB)]
                )
                for j, c in order:
                    nc.tensor.transpose(
                        psts[c][:, j * 128 : (j + 1) * 128],
                        x_nat[:, j, c * 128 : (c + 1) * 128],
                        ident[:],
                    )
                xt = []
                for c in range(EC):
                    t = xp.tile([128, NBS], F16, tag=f"xt{c}")
                    nc.vector.tensor_copy(out=t, in_=psts[c].bitcast(F32))
                    xt.append(t)
                return xt

            def conv16(x_nat):
                """fp16 convert of an x chunk (ACT+DVE halves). Emitted right
                after scores(k) so the DVE half clears its queue before the
                wT evictions that chunk k's att matmuls wait on."""
                x16 = xp.tile([128, NB, E], F16, tag="x16")
                nc.scalar.copy(
                    out=x16[:, 0:2, :], in_=x_nat[:, 0:2, :].bitcast(F32)
                )
                nc.vector.tensor_copy(
                    out=x16[:, 2:4, :], in_=x_nat[:, 2:4, :].bitcast(F32)
                )
                return x16

            def trans16(x16):
                """PE transpose of a pre-converted fp16 x chunk -> xT tiles."""
                xt = []
                for c in range(EC):
                    pst = ps.tile([128, NBS], F16, tag="ps")
                    for j in range(NB):
                        nc.tensor.transpose(
                            pst[:, j * 128 : (j + 1) * 128],
                            x16[:, j, c * 128 : (c + 1) * 128],
                            ident_f16[:],
                        )
                    t = xp.tile([128, NBS], F16, tag=f"xt{c}")
                    nc.scalar.copy(out=t, in_=pst)
                    xt.append(t)
                return xt

            def proj(xt):
                """QT/KT/V projections from xT (fp16 operands, fp32 PSUM)."""
                qt, kt = [], []
                for h in range(H):
                    p = ps.tile([128, NBS], F32, tag="ps")
                    for c in range(EC):
                        nc.tensor.matmul(
                            p,
                            w16["q"][:, c, h * 128 : (h + 1) * 128],
                            xt[c],
                            start=(c == 0),
                            stop=(c == EC - 1),
                        )
                    t = qkv.tile([128, NBS], F16, tag=f"qt{h}")
                    nc.scalar.add(out=t, in_=p, add=bq_sb[:, h : h + 1])
                    qt.append(t)
                    p = ps.tile([128, NBS], F32, tag="ps")
                    for c in range(EC):
                        nc.tensor.matmul(
                            p,
                            w16["k"][:, c, h * 128 : (h + 1) * 128],
                            xt[c],
                            start=(c == 0),
                            stop=(c == EC - 1),
                        )
                    t = qkv.tile([128, NBS], F16, tag=f"kt{h}")
                    nc.scalar.add(out=t, in_=p, add=bk_sb[:, h : h + 1])
                    kt.append(t)
                v_sb = []
                for j in range(NB):
                    p = ps.tile([128, E], F32, tag="ps")
                    for c in range(EC):
                        nc.tensor.matmul(
                            p,
                            xt[c][:, j * 128 : (j + 1) * 128],
                            w16["v"][:, c, :],
                            start=(c == 0),
                            stop=(c == EC - 1),
                        )
                    t = qkv.tile([128, E], BF16, tag=f"v{j}")
                    nc.vector.tensor_add(out=t, in0=p, in1=bv_sb)
                    v_sb.append(t)
                return qt, kt, v_sb

            def attn_scores(qt, kt, js=None, act_norms=False):
                """scores + softmax (no max-subtraction) -> normalized bf16 w."""
                wns = []
                for j in (range(NB) if js is None else js):
                    ps_s = ps.tile([128, H, 128], F32, tag="ps")
                    for h in range(H):
                        nc.tensor.matmul(
                            ps_s[:, h, :],
                            qt[h][:, j * 128 : (j + 1) * 128],
                            kt[h][:, j * 128 : (j + 1) * 128],
                            start=True,
                            stop=True,
                        )
                    w_exp = wsm.tile([128, H, 128], BF16, tag="wexp")
                    nc.scalar.activation(
                        out=w_exp,
                        in_=ps_s,
                        func=mybir.ActivationFunctionType.Exp,
                        bias=0.0,
                        scale=1.0,
                    )
                    sumexp = stats.tile([128, H], F32, tag="sumexp")
                    nc.vector.reduce_sum(
                        out=sumexp, in_=w_exp, axis=mybir.AxisListType.X
                    )
                    recip = stats.tile([128, H], F32, tag="recip")
                    nc.vector.reciprocal(out=recip, in_=sumexp)
                    wn = wsm.tile([128, H, 128], BF16, tag="wn")
                    for h in range(H):
                        if act_norms and h < 2:
                            nc.scalar.activation(
                                out=wn[:, h, :],
                                in_=w_exp[:, h, :],
                                func=mybir.ActivationFunctionType.Identity,
                                bias=0.0,
                                scale=recip[:, h : h + 1],
                            )
                        else:
                            nc.vector.tensor_scalar_mul(
                                out=wn[:, h, :],
                                in0=w_exp[:, h, :],
                                scalar1=recip[:, h : h + 1],
                            )
                    wns.append(wn)
                return wns

            def tail_watt(chunk, wns, v_sb, js=None):
                """wT transposes + att = v.T-form matmuls -> at fp16 tiles."""
                js = range(NB) if js is None else js
                wn_of = wns if callable(wns) else lambda j: wns[j]
                wt_sbs = {}
                for j in js:
                    ps_wt = ps.tile([128, H, 128], BF16, tag="ps")
                    for h in range(H):
                        nc.tensor.transpose(
                            ps_wt[:, h, :], wn_of(j)[:, h, :], ident_bf[:]
                        )
                    wt_sb = wsm.tile([128, H, 128], BF16, tag=f"wt{j}")
                    nc.vector.tensor_copy(out=wt_sb, in_=ps_wt)
                    wt_sbs[j] = wt_sb
                ats = {}
                for j in js:
                    ps_at = ps.tile([128, H, 128], F32, tag="ps")
                    for h in range(H):
                        nc.tensor.matmul(
                            ps_at[:, h, :],
                            v_sb[j][:, h * 128 : (h + 1) * 128],
                            wt_sbs[j][:, h, :],
                            start=True,
                            stop=True,
                        )
                    at = attn.tile([128, H, 128], F16, tag=f"at{j}")
                    nc.scalar.copy(out=at, in_=ps_at)
                    ats[j] = at
                return ats

            def tail_o(chunk, ats, js=None, o_sb=None):
                """O projection + bias + store."""
                b0 = chunk * NB
                js = range(NB) if js is None else js
                split_store = chunk == NCHUNK - 1
                if o_sb is None:
                    o_sb = attn.tile([128, NB, E], F32, tag="o")
                for j in js:
                    p = ps.tile([128, E], F32, tag="ps")
                    for h in range(H):
                        nc.tensor.matmul(
                            p,
                            ats[j][:, h, :],
                            w16["o"][:, h, :],
                            start=(h == 0),
                            stop=(h == H - 1),
                        )
                    nc.vector.tensor_add(out=o_sb[:, j, :], in0=p, in1=bo_sb)
                    if split_store:
                        eng = nc.sync if j % 2 == 0 else nc.scalar
                        eng.dma_start(out=out[b0 + j], in_=o_sb[:, j, :])
                if not split_store and (NB - 1) in js:
                    nc.sync.dma_start(
                        out=bass.AP(
                            tensor=out.tensor,
                            offset=b0 * S * E,
                            ap=[[E, 128], [S * E, NB], [1, E]],
                        ),
                        in_=o_sb,
                    )
                return o_sb

            # Startup: x0/x1 dispatch on the SP queue, weights+biases on the
            # ACT queue (emitted first so their dispatch overlaps the x DMA),
            # then the software pipeline:
            #   scores(k) | projections(k+1) | transposes(k+2) | tail(k)
            x0 = emit_x_dma(0, fine=True)
            x1 = emit_x_dma(1)
            load_weights()
            load_biases()
            xts = {0: conv_trans0(x0)}
            states = {0: proj(xts[0])}
            xts[1] = conv_trans0(x1, by_batch=False)
            wns = {}
            for k in range(NCHUNK - 1):
                wns[k] = attn_scores(states[k][0], states[k][1])
                if k + 2 < NCHUNK:
                    x16_next = conv16(emit_x_dma(k + 2))
                states[k + 1] = proj(xts[k + 1])
                ats = tail_watt(k, wns[k], states[k][2])
                if k + 2 < NCHUNK:
                    xts[k + 2] = trans16(x16_next)
                if k < NCHUNK - 2:
                    tail_o(k, ats)
                else:
                    o_prev = tail_o(k, ats, js=[0, 1])
                    ats_prev = ats
            kl = NCHUNK - 1
            wns[kl] = attn_scores(states[kl][0], states[kl][1], act_norms=True)
            tail_o(kl - 1, ats_prev, js=[2, 3], o_sb=o_prev)
            ats = tail_watt(kl, wns[kl], states[kl][2])
            tail_o(kl, ats)

    nc.compile()
    return nc


def kernel(**inputs):
    if "nc" not in _CACHE:
        _CACHE["nc"] = build()
    nc = _CACHE["nc"]

    x = np.ascontiguousarray(np.asarray(inputs["x"], dtype=np.float32))
    shared = {
        "Wq16": np.ascontiguousarray(np.asarray(inputs["Wq"], dtype=np.float16)),
        "Wk16": np.ascontiguousarray(np.asarray(inputs["Wk"], dtype=np.float16)),
        "Wv16": np.ascontiguousarray(np.asarray(inputs["Wv"], dtype=np.float16)),
        "Wo16": np.ascontiguousarray(np.asarray(inputs["Wo"], dtype=np.float16)),
        "bqp": np.ascontiguousarray(
            np.asarray(inputs["bq"], dtype=np.float32).reshape(EC, 128).T
        ),
        "bkp": np.ascontiguousarray(
            np.asarray(inputs["bk"], dtype=np.float32).reshape(EC, 128).T
        ),
        "bv": np.ascontiguousarray(np.asarray(inputs["bv"], dtype=np.float32)),
        "bo": np.ascontiguousarray(np.asarray(inputs["bo"], dtype=np.float32)),
    }
    in_maps = [
        {"x": x[i * BLOC : (i + 1) * BLOC], **shared} for i in range(NCORES)
    ]
    res = run_bass_kernel_spmd(nc, in_maps, core_ids=list(range(NCORES)))
    return np.concatenate([res.results[i]["out"] for i in range(NCORES)], axis=0)
